# revision 9
# baseline (speedup 1.0000x reference)
"""MLA q/kv projection kernel for Trainium2, 8 NeuronCores, SPMD data-parallel
over the token dimension (512 tokens per core). v2: transpose-free.

Per-core pipeline:
  A (kv):  ps_kv[m][128,512] = x.T-tiles @ wkv_latent  (token-major),
           ps_kr[64,512] = wkv_rope.T @ x.T (rope-major, PE-transposed back);
           ckv rmsnorm*gamma + k_rope rope -> kv_sb -> one DMA.
  B (cq):  cT[j][128c,512t] = wq_a[k,j].T @ xt[k]  -- the down-projection is
           computed directly TRANSPOSED (c-major), so mm2's stationary operand
           needs no transpose at all.  ACT squares each cT tile; a ones-vector
           matmul accumulates sumsq[1,512] over j; 4 tiny K=1 matmuls
           transpose it to token-major; r = rsqrt(ms+eps) is applied at mm2
           OUTPUT time (per-token scalar commutes through the matmul since
           gamma_cq is folded into wq_b host-side).
  C (mm2): q[128t,512] = cT.T-tiles @ wb_n; nope tiles: ACT psum*r -> bf16;
           rope tiles: DVE rope with r-prescaled cos/sin; one DMA per n-tile.
  Rope n-tiles are interleaved early/mid so the kernel tail is a cheap nope
  store.  Weight DMAs ride the SP HWDGE queue; xt/cs/outputs ride the ACT
  queue.  Output is bf16 (host upcasts).
"""

import os

import numpy as np

import concourse.bass as bass
import concourse.tile as tile
from concourse import mybir
from concourse.bass_utils import run_bass_kernel_spmd
from concourse.vector_clock import ScopedClock, VectorClock

F32 = mybir.dt.float32
BF16 = mybir.dt.bfloat16

N_CORES = 8
T = 4096
TC = T // N_CORES           # 512 tokens per core
MT = TC // 128              # 4 token tiles
H = 7168
KH = H // 128               # 56 contraction tiles for mm1
L = 1536                    # q latent
KL = L // 128               # 12 c-dim tiles
KV_RANK = 512
R = 64                      # rope dims
N_HEADS = 128
QK_NOPE = 128
DN = N_HEADS * (QK_NOPE + R)   # 24576
NT2 = DN // 512                # 48 n-tiles for mm2
NOPE_TILES = N_HEADS * QK_NOPE // 512   # 32 (4 heads each)
ROPE_TILES = NT2 - NOPE_TILES           # 16 (8 heads each)
OUTW = DN + KV_RANK + R     # 25152
EPS = 1e-6

KC = 8                      # k-chunks for phase A DMA pipelining
KCL = KH // KC              # 7 k-tiles per chunk
JP = 6                      # j-pairs for phase B
KCB = 4                     # k-chunks per j-pair DMA
KBL = KH // KCB             # 14 k-tiles per B chunk


def split_multi_waits(nc, limit=1):
    """Walrus accepts at most one sync-wait per TPB instruction; hoist extra
    waits onto single-wait NoOps on the same engine."""
    skip = (mybir.InstAllEngineBarrier, mybir.InstEventSemaphore)
    for f in nc.m.functions:
        for bb in f.blocks:
            new_insts = []
            changed = False
            for inst in bb.instructions:
                si = inst.sync_info
                waits = list(si.on_wait) if si is not None and si.on_wait else []
                if len(waits) > limit and not isinstance(inst, skip):
                    for w in waits[:-limit]:
                        nop = mybir.InstNoOp(
                            name=nc.get_next_instruction_name(),
                            sync_info=mybir.SyncInfo(on_wait=[w], on_update=[]),
                            bass_nofuse=True,
                            engine=inst.engine,
                        )
                        new_insts.append(nop)
                    inst.sync_info = mybir.SyncInfo(
                        on_wait=waits[-limit:], on_update=list(si.on_update))
                    changed = True
                new_insts.append(inst)
            if changed:
                bb.instructions = new_insts
    return nc


class PatchedTC(tile.TileContext):
    """The SP Drain only accepts ONE sync-wait; chain single-wait drains."""

    def _drain_and_barrier(self, tick_clock, wait_clock):
        nc = self.nc
        gc = tick_clock.global_clock
        nprocs = len(gc)
        procs = [p for p in range(nprocs) if gc[p] > 0] or [0]
        for p in procs:
            d = nc.sync.drain()
            vc = VectorClock([0] * nprocs)
            vc.require_at_least(p, gc[p])
            wait_clock.add_sem_waits(d.ins, ScopedClock({None: vc}))
        nc.all_engine_barrier()
        assert self.sems is not None
        popped = nc._tile_sem_poison_stack.pop()
        assert popped is self._sem_poison
        nc.clear_and_free_semaphores(list(self.sems.allocated().values()))
        nc.all_engine_barrier()


def mm2_order():
    """n-tile schedule: a couple of nope tiles first (r not yet needed),
    rope tiles interleaved mid-stream, nope-only tail."""
    order = [("nope", 0), ("nope", 1)]
    ni, ri = 2, 0
    while ri < ROPE_TILES:
        order.append(("rope", ri)); ri += 1
        if ni < NOPE_TILES:
            order.append(("nope", ni)); ni += 1
    while ni < NOPE_TILES:
        order.append(("nope", ni)); ni += 1
    return order


def build_nc(split=True):
    reps = int(os.environ.get("MLA_REPS", "1"))
    phases = os.environ.get("MLA_PHASES", "ABC")
    wb_bufs = int(os.environ.get("MLA_WB_BUFS", "3"))
    wqa_bufs = int(os.environ.get("MLA_WQA_BUFS", "3"))
    wkv_bufs = int(os.environ.get("MLA_WKV_BUFS", "6"))
    qout_bufs = int(os.environ.get("MLA_QOUT_BUFS", "2"))
    sq_bufs = int(os.environ.get("MLA_SQ_BUFS", "3"))

    nc = bass.Bass()
    xt = nc.dram_tensor("xt", [H, TC], BF16, kind="ExternalInput")
    wqa = nc.dram_tensor("wqa", [JP, KCB, 128, 2, KBL, 128], BF16,
                         kind="ExternalInput")
    wkv = nc.dram_tensor("wkv", [H, KV_RANK], BF16, kind="ExternalInput")
    wkr = nc.dram_tensor("wkr", [128, KH, R], BF16, kind="ExternalInput")
    wb = nc.dram_tensor("wb", [NT2, 128, KL, 512], BF16, kind="ExternalInput")
    cs = nc.dram_tensor("cs", [TC, 1024], F32, kind="ExternalInput")
    gkv = nc.dram_tensor("gkv", [KV_RANK], F32, kind="ExternalInput")
    # grouped output layout: [nope h*128 | rope h*64 | ckv | krope]; the host
    # re-interleaves per-head (nope|rope) afterwards.
    out = nc.dram_tensor("out", [TC, OUTW], BF16, kind="ExternalOutput")

    out_ap = out.ap()
    NOPE_W = N_HEADS * QK_NOPE   # 16384

    with PatchedTC(nc) as tc:
        with (
            tc.tile_pool(name="consts", bufs=1) as p_const,
            tc.tile_pool(name="cs", bufs=1) as p_cs,
            tc.tile_pool(name="cqT", bufs=1) as p_cqT,
            tc.tile_pool(name="xt", bufs=1) as p_xt,
            tc.tile_pool(name="wkv", bufs=wkv_bufs) as p_wkv,
            tc.tile_pool(name="wqa", bufs=wqa_bufs) as p_wqa,
            tc.tile_pool(name="sq", bufs=sq_bufs) as p_sq,
            tc.tile_pool(name="scr", bufs=1) as p_scr,
            tc.tile_pool(name="stats", bufs=1) as p_stats,
            tc.tile_pool(name="tmp", bufs=2) as p_tmp,
            tc.tile_pool(name="wb", bufs=wb_bufs) as p_wb,
            tc.tile_pool(name="qout", bufs=qout_bufs) as p_qout,
            tc.tile_pool(name="psum", bufs=7, space="PSUM") as p_ps,
            tc.tile_pool(name="psacc", bufs=1, space="PSUM") as p_pacc,
        ):
            # ---- constants (loaded once, off the critical path) ----
            eps_t = p_const.tile([128, 1], F32, tag="eps", name="eps_t")
            nc.vector.memset(eps_t, EPS)
            ones_b = p_const.tile([128, 1], BF16, tag="ones", name="ones_b")
            nc.vector.memset(ones_b, 1.0)
            one_f = p_const.tile([1, 1], F32, tag="onef", name="one_f")
            nc.vector.memset(one_f, 1.0)
            gamma_b = p_const.tile([128, KV_RANK], F32, tag="gamma",
                                   name="gamma_b")

            for _rep in range(reps):
                # ---- phase A: kv path (token-major) + xt residency ----
                # chunk schedule: tiny first chunk so PE starts ~1.5us in
                chunks = [(0, 1), (1, 3)] + [(4 + 4 * i, 4)
                                          for i in range(13)]
                xt_sb = p_xt.tile([128, KH, TC], BF16, tag="xt", name="xt_sb")
                kr_sb = p_xt.tile([128, KH, R], BF16, tag="wkr", name="kr_sb")

                ps_kv = [p_ps.tile([128, 512], F32, tag="ps", name="ps")
                         for _ in range(MT)]
                ps_kr = p_ps.tile([128, 512], F32, tag="ps", name="ps")
                wkv_t = {}
                for c, (k0, kn) in enumerate(chunks):
                    if "A" in phases or "B" in phases:
                        nc.scalar.dma_start(
                            out=xt_sb[:, k0:k0 + kn, :],
                            in_=xt.ap()[k0 * 128:(k0 + kn) * 128, :]
                            .rearrange("(k p) t -> p k t", p=128))
                    if "A" not in phases:
                        continue
                    wkv_t[c] = p_wkv.tile([128, 4, KV_RANK], BF16,
                                          tag="wkv", name="wkv_t")
                    nc.sync.dma_start(
                        out=wkv_t[c][:, 0:kn, :],
                        in_=wkv.ap()[k0 * 128:(k0 + kn) * 128, :]
                        .rearrange("(k p) c -> p k c", p=128))
                    nc.sync.dma_start(
                        out=kr_sb[:, k0:k0 + kn, :],
                        in_=wkr.ap()[:, k0:k0 + kn, :])
                    for kl in range(kn):
                        k = k0 + kl
                        for m in range(MT):
                            nc.tensor.matmul(
                                ps_kv[m],
                                lhsT=xt_sb[:, k, m * 128:(m + 1) * 128],
                                rhs=wkv_t[c][:, kl, :],
                                start=(k == 0), stop=(k == KH - 1))
                            # k_rope token-major, same stationary operand;
                            # all 4 m-accumulators share ps_kr via
                            # per-element has_written (start clears the
                            # bank only once, at k==0 m==0)
                            nc.tensor.matmul(
                                ps_kr[:, m * R:(m + 1) * R],
                                lhsT=xt_sb[:, k, m * 128:(m + 1) * 128],
                                rhs=kr_sb[:, k, :],
                                start=(k == 0 and m == 0),
                                stop=(k == KH - 1))
                # cs / gamma ride behind the phase-A input stream (needed
                # only at kv-norm time)
                cs_sb = []
                for m in range(MT):
                    t = p_cs.tile([128, 1024], F32, tag=f"cs{m}",
                                  name=f"cs{m}")
                    nc.scalar.dma_start(
                        out=t, in_=cs.ap()[m * 128:(m + 1) * 128, :])
                    cs_sb.append(t)
                if _rep == 0:
                    g_ap = gkv.ap()
                    nc.sync.dma_start(
                        out=gamma_b,
                        in_=bass.AP(tensor=g_ap.tensor, offset=g_ap.offset,
                                    ap=[[0, 128]] + [list(p) for p in
                                                     g_ap.ap]),
                    )

                # kv norm + krope rope -> kv_sb -> one DMA
                kv_sb = p_scr.tile([128, MT, KV_RANK + R], BF16, tag="kv",
                                   name="kv_sb")
                for m in range(MT) if "A" in phases else []:
                    scr = p_tmp.tile([128, KV_RANK], BF16, tag="sqkv",
                                     name="sqkv")
                    st = p_stats.tile([128, 1], F32, tag=f"stk{m}",
                                      name=f"stk{m}")
                    nc.scalar.activation(
                        out=scr, in_=ps_kv[m],
                        func=mybir.ActivationFunctionType.Square,
                        accum_out=st)
                    nc.scalar.activation(
                        out=st, in_=st,
                        func=mybir.ActivationFunctionType.Sqrt,
                        bias=eps_t, scale=1.0 / KV_RANK)
                    nc.vector.reciprocal(out=st, in_=st)
                    nc.vector.tensor_scalar_mul(
                        out=kv_sb[:, m, 0:KV_RANK], in0=ps_kv[m], scalar1=st)
                    nc.vector.tensor_mul(
                        out=kv_sb[:, m, 0:KV_RANK],
                        in0=kv_sb[:, m, 0:KV_RANK], in1=gamma_b)
                    x1 = ps_kr[:, m * R:m * R + 32]
                    x2 = ps_kr[:, m * R + 32:m * R + 64]
                    ta = p_tmp.tile([128, 64], F32, tag="ta", name="ta")
                    tb = p_tmp.tile([128, 64], F32, tag="tb", name="tb")
                    nc.vector.tensor_mul(out=ta[:, 0:32], in0=x1,
                                         in1=cs_sb[m][:, 0:32])
                    nc.vector.tensor_mul(out=tb[:, 0:32], in0=x2,
                                         in1=cs_sb[m][:, 512:544])
                    nc.vector.tensor_sub(
                        out=kv_sb[:, m, KV_RANK:KV_RANK + 32],
                        in0=ta[:, 0:32], in1=tb[:, 0:32])
                    nc.vector.tensor_mul(out=ta[:, 32:64], in0=x2,
                                         in1=cs_sb[m][:, 32:64])
                    nc.vector.tensor_mul(out=tb[:, 32:64], in0=x1,
                                         in1=cs_sb[m][:, 544:576])
                    nc.vector.tensor_add(
                        out=kv_sb[:, m, KV_RANK + 32:KV_RANK + 64],
                        in0=ta[:, 32:64], in1=tb[:, 32:64])
                if "A" in phases:
                    nc.scalar.dma_start(
                        out=out_ap[:, DN:OUTW].rearrange("(m p) c -> p m c",
                                                         p=128),
                        in_=kv_sb)

                # ---- phase B: cq down-proj, directly transposed ----
                cqT = p_cqT.tile([128, KL, TC], BF16, tag="cqT", name="cqT")
                ps_ss = p_pacc.tile([128, 512], F32, tag="pss", name="pss")
                if "B" not in phases and "C" in phases:
                    nc.vector.memset(cqT, 0.25)
                for jp in range(JP) if "B" in phases else []:
                    ps_c = [p_ps.tile([128, 512], F32, tag="ps", name="ps")
                            for _ in range(2)]
                    wqa_t = {}
                    for cb in range(KCB):
                        wqa_t[cb] = p_wqa.tile([128, 2, KBL, 128], BF16,
                                               tag="wqa", name="wqa_t")
                        nc.sync.dma_start(out=wqa_t[cb],
                                          in_=wqa.ap()[jp, cb])
                        for j2 in range(2):
                            for kl in range(KBL):
                                k = cb * KBL + kl
                                nc.tensor.matmul(
                                    ps_c[j2],
                                    lhsT=wqa_t[cb][:, j2, kl, :],
                                    rhs=xt_sb[:, k, :],
                                    start=(k == 0), stop=(k == KH - 1))
                    for j2 in range(2):
                        j = jp * 2 + j2
                        nc.vector.tensor_copy(out=cqT[:, j, :], in_=ps_c[j2])
                        sq = p_sq.tile([128, 512], BF16, tag="sq", name="sq")
                        nc.scalar.activation(
                            out=sq, in_=ps_c[j2],
                            func=mybir.ActivationFunctionType.Square)
                        nc.tensor.matmul(
                            ps_ss[0:1, :], lhsT=ones_b, rhs=sq,
                            start=(j == 0), stop=(j == KL - 1))

                # r = rsqrt(mean + eps), token-major via 4 tiny transposes
                if "B" not in phases:
                    nc.vector.memset(ps_ss[0:1, :], 1.0)
                ss_row = p_scr.tile([1, 512], F32, tag="ssrow", name="ss_row")
                nc.vector.tensor_copy(out=ss_row, in_=ps_ss[0:1, :])
                ps_r = p_pacc.tile([128, 512], F32, tag="pss", name="ps_r")
                for m in range(MT):
                    nc.tensor.matmul(
                        ps_r[:, m:m + 1],
                        lhsT=ss_row[0:1, m * 128:(m + 1) * 128],
                        rhs=one_f,
                        start=(m == 0), stop=(m == MT - 1))
                r_sb = p_stats.tile([128, MT], F32, tag="rsb", name="r_sb")
                nc.scalar.activation(
                    out=r_sb, in_=ps_r[:, 0:MT],
                    func=mybir.ActivationFunctionType.Sqrt,
                    bias=eps_t, scale=1.0 / L)
                nc.vector.reciprocal(out=r_sb, in_=r_sb)
                # pre-scale cos/sin by r for the q-rope tiles (in place; cs
                # was already consumed by the kv path above)
                for m in range(MT):
                    nc.vector.tensor_scalar_mul(
                        out=cs_sb[m], in0=cs_sb[m], scalar1=r_sb[:, m:m + 1])

                # ---- phase C: mm2 ----
                order = mm2_order() if "C" in phases else []
                for oi, (kind, i) in enumerate(order):
                    n = i if kind == "nope" else NOPE_TILES + i
                    last = oi == len(order) - 1
                    wb_t = p_wb.tile([128, KL, 512], BF16, tag="wb",
                                     name="wb_t")
                    nc.sync.dma_start(out=wb_t, in_=wb.ap()[n])
                    ps_t = [p_ps.tile([128, 512], F32, tag="ps", name="ps")
                            for _ in range(MT)]
                    if last:
                        # m-outer so each token tile finishes (and stores)
                        # early -- cheap kernel tail
                        q_sb = p_qout.tile([128, MT, 512], BF16, tag="q",
                                           name="q_sb")
                        for m in range(MT):
                            for k in range(KL):
                                nc.tensor.matmul(
                                    ps_t[m],
                                    lhsT=cqT[:, k, m * 128:(m + 1) * 128],
                                    rhs=wb_t[:, k, :],
                                    start=(k == 0), stop=(k == KL - 1))
                            nc.scalar.mul(out=q_sb[:, m, :], in_=ps_t[m],
                                          mul=r_sb[:, m:m + 1])
                            nc.scalar.dma_start(
                                out=out_ap[m * 128:(m + 1) * 128,
                                           i * 512:(i + 1) * 512],
                                in_=q_sb[:, m, :])
                        continue
                    for k in range(KL):
                        for m in range(MT):
                            nc.tensor.matmul(
                                ps_t[m],
                                lhsT=cqT[:, k, m * 128:(m + 1) * 128],
                                rhs=wb_t[:, k, :],
                                start=(k == 0), stop=(k == KL - 1))
                    q_sb = p_qout.tile([128, MT, 512], BF16, tag="q",
                                       name="q_sb")
                    if kind == "nope":
                        for m in range(MT):
                            nc.scalar.mul(out=q_sb[:, m, :], in_=ps_t[m],
                                          mul=r_sb[:, m:m + 1])
                        nc.scalar.dma_start(
                            out=out_ap[:, i * 512:(i + 1) * 512].rearrange(
                                "(m p) c -> p m c", p=128),
                            in_=q_sb)
                    else:
                        for m in range(MT):
                            x = ps_t[m].rearrange("p (h d) -> p h d", d=R)
                            qo = q_sb[:, m, :].rearrange("p (h d) -> p h d",
                                                         d=R)
                            cosr = cs_sb[m][:, 0:512].rearrange(
                                "p (h d) -> p h d", d=R)
                            sinr = cs_sb[m][:, 512:1024].rearrange(
                                "p (h d) -> p h d", d=R)
                            ta = p_tmp.tile([128, 256], F32, tag="tc",
                                            name="tc")
                            tb = p_tmp.tile([128, 256], F32, tag="td",
                                            name="td")
                            va = ta.rearrange("p (h d) -> p h d", d=32)
                            vb = tb.rearrange("p (h d) -> p h d", d=32)
                            nc.vector.tensor_mul(
                                out=va, in0=x[:, :, 0:32],
                                in1=cosr[:, :, 0:32])
                            nc.vector.tensor_mul(
                                out=vb, in0=x[:, :, 32:64],
                                in1=sinr[:, :, 0:32])
                            nc.vector.tensor_sub(
                                out=qo[:, :, 0:32], in0=va, in1=vb)
                            ta2 = p_tmp.tile([128, 256], F32, tag="tc",
                                             name="tc")
                            tb2 = p_tmp.tile([128, 256], F32, tag="td",
                                             name="td")
                            va2 = ta2.rearrange("p (h d) -> p h d", d=32)
                            vb2 = tb2.rearrange("p (h d) -> p h d", d=32)
                            nc.vector.tensor_mul(
                                out=va2, in0=x[:, :, 32:64],
                                in1=cosr[:, :, 32:64])
                            nc.vector.tensor_mul(
                                out=vb2, in0=x[:, :, 0:32],
                                in1=sinr[:, :, 32:64])
                            nc.vector.tensor_add(
                                out=qo[:, :, 32:64], in0=va2, in1=vb2)
                        nc.scalar.dma_start(
                            out=out_ap[:, NOPE_W + i * 512:
                                       NOPE_W + (i + 1) * 512].rearrange(
                                "(m p) c -> p m c", p=128),
                            in_=q_sb)
    if split:
        split_multi_waits(nc)
    return nc


def prep_inputs(token_x, wq_a, wq_b, wkv, rope_cos, rope_sin, gamma_cq,
                gamma_ckv):
    """Host-side sharding + layout prep. Returns in_maps for the 8 cores."""
    bf16 = mybir.dt.np(BF16)
    # wq_a [H, L] -> [jp, kcb, p, j2, kbl, c]
    wqa = np.ascontiguousarray(
        wq_a.reshape(KCB, KBL, 128, JP, 2, 128)
        .transpose(3, 0, 2, 4, 1, 5)).astype(bf16)
    wkv_l = np.ascontiguousarray(wkv[:, :KV_RANK]).astype(bf16)
    wkr = np.ascontiguousarray(
        wkv[:, KV_RANK:].reshape(KH, 128, R).transpose(1, 0, 2)).astype(bf16)
    # wq_b: fold gamma_cq, group columns nope-first / rope-last, then
    # [n, p, k, c]
    wbs = (wq_b.astype(np.float32)
           * gamma_cq.astype(np.float32)[:, None]).reshape(L, N_HEADS,
                                                           QK_NOPE + R)
    wb_re = np.concatenate(
        [wbs[:, :, :QK_NOPE].reshape(L, -1),
         wbs[:, :, QK_NOPE:].reshape(L, -1)], axis=1)      # [L, 24576]
    wb = np.ascontiguousarray(
        wb_re.reshape(KL, 128, NT2, 512).transpose(2, 1, 0, 3)).astype(bf16)
    gkv = np.ascontiguousarray(gamma_ckv.astype(np.float32))
    in_maps = []
    for c in range(N_CORES):
        sl = slice(c * TC, (c + 1) * TC)
        xt = np.ascontiguousarray(token_x[sl].T).astype(bf16)
        cos_rep = np.tile(rope_cos[sl].astype(np.float32), (1, 8))
        sin_rep = np.tile(rope_sin[sl].astype(np.float32), (1, 8))
        cs = np.ascontiguousarray(
            np.concatenate([cos_rep, sin_rep], axis=1))    # [TC, 1024]
        in_maps.append({"xt": xt, "wqa": wqa, "wkv": wkv_l, "wkr": wkr,
                        "wb": wb, "cs": cs, "gkv": gkv})
    return in_maps


def postprocess(grouped):
    """Un-group the device output: grouped [T, OUTW] bf16/f32 -> final f32."""
    grouped = np.asarray(grouped).astype(np.float32).reshape(-1, OUTW)
    t = grouped.shape[0]
    NOPE_W = N_HEADS * QK_NOPE
    q = np.concatenate(
        [grouped[:, :NOPE_W].reshape(t, N_HEADS, QK_NOPE),
         grouped[:, NOPE_W:DN].reshape(t, N_HEADS, R)],
        axis=2).reshape(t, DN)
    return np.concatenate([q, grouped[:, DN:]], axis=1)


def kernel(token_x, wq_a, wq_b, wkv, rope_cos, rope_sin, gamma_cq, gamma_ckv):
    token_x, wq_a, wq_b, wkv, rope_cos, rope_sin, gamma_cq, gamma_ckv = (
        np.asarray(a) for a in (token_x, wq_a, wq_b, wkv, rope_cos, rope_sin,
                                gamma_cq, gamma_ckv))
    in_maps = prep_inputs(token_x, wq_a, wq_b, wkv, rope_cos, rope_sin,
                          gamma_cq, gamma_ckv)
    nc = build_nc()
    res = run_bass_kernel_spmd(nc, in_maps, list(range(N_CORES)))
    grouped = np.concatenate(
        [np.asarray(res.results[c]["out"]) for c in range(N_CORES)], axis=0)
    return postprocess(grouped)
